# revision 10
# baseline (speedup 1.0000x reference)
"""Bass/Tile kernel for nn_Attention_41532333753073 on 8 trn2 NeuronCores.

Sharding: core i -> (batch b = i//4, head-group g = i%4) with 8 heads each.
Per core: QKV projections (bf16 matmuls, fp32 psum), RoPE, causal attention in
S^T layout (k_t on partitions -> softmax row-sum via ones-column in the PV
matmul, no transposes), output projection producing a partial [T, DM] that the
host sums over the 4 head-groups per batch.
"""

import numpy as np
import ml_dtypes

BF16 = ml_dtypes.bfloat16

# Problem shapes (hardcoded)
B, T, DM = 2, 2048, 2048
H, DH = 32, 64
GROUPS = 4
HG = H // GROUPS          # 8 heads per group
DG = HG * DH              # 512 columns per group
P = 128
KT = DM // P              # 16 contraction tiles
NT = T // P               # 16 t tiles
NHP = HG // 2             # 4 head pairs
NQC = T // 512            # 4 q chunks of 512
ROPE_THETA = 10000.0
N_CORES = 8
SCALE = 1.0 / np.sqrt(DH)


def _host_constants():
    inv = 1.0 / (ROPE_THETA ** (np.arange(0, DH, 2, dtype=np.float64) / DH))  # [32]
    t = np.arange(T, dtype=np.float64)
    f = np.outer(inv, t)                       # [32, T]
    cos = np.cos(f).astype(np.float32)
    sin = np.sin(f).astype(np.float32)
    cosT = np.tile(cos, (4, 1))                # [128, T]
    sgn = np.repeat(np.array([-1.0, 1.0, -1.0, 1.0], np.float32), 32)[:, None]
    sinT = np.tile(sin, (4, 1)) * sgn          # [128, T] signed
    i = np.arange(P)[:, None]
    j = np.arange(512)[None, :]
    mask = np.stack([(j - i >= 128 * d) for d in range(4)], axis=1)  # [128,4,512]
    return (cosT.astype(BF16), sinT.astype(BF16), mask.astype(BF16))


_CONSTS = _host_constants()


def _build_bass():
    from contextlib import ExitStack

    import concourse.bass as bass
    import concourse.bacc as bacc
    import concourse.tile as tile
    from concourse import mybir

    BF = mybir.dt.bfloat16
    F32 = mybir.dt.float32
    AF = mybir.ActivationFunctionType

    nc = bacc.Bacc(None, target_bir_lowering=False)
    hidT = nc.declare_dram_parameter("hidT", [DM, T], BF, isOutput=False)
    wq = nc.declare_dram_parameter("wq", [DM, DG], BF, isOutput=False)
    wk = nc.declare_dram_parameter("wk", [DM, DG], BF, isOutput=False)
    wv = nc.declare_dram_parameter("wv", [DM, DG], BF, isOutput=False)
    v1s = nc.declare_dram_parameter("v1s", [T, DG], BF, isOutput=False)
    cosT = nc.declare_dram_parameter("cosT", [P, T], BF, isOutput=False)
    sinT = nc.declare_dram_parameter("sinT", [P, T], BF, isOutput=False)
    maskp = nc.declare_dram_parameter("mask", [P, 4, 512], BF, isOutput=False)
    wo = nc.declare_dram_parameter("wo", [DG, DM], BF, isOutput=False)
    out = nc.declare_dram_parameter("out", [T, DM], F32, isOutput=True)

    hid_r = hidT[:].rearrange("(k p) t -> p k t", p=P)     # [128,16,2048]
    wq_r = wq[:].rearrange("(k p) c -> p k c", p=P)        # [128,16,512]
    wk_r = wk[:].rearrange("(k p) c -> p k c", p=P)
    wv_r = wv[:].rearrange("(k p) c -> p k c", p=P)
    v1_r = v1s[:].rearrange("(n p) c -> p n c", p=P)       # [128,16,512]
    wo_r = wo[:].rearrange("(k p) n -> p k n", p=P)        # [128,4,2048]
    out_r = out[:].rearrange("(m p) n -> p m n", p=P)      # [128,16,2048]

    SWAP_SRC = [1, 0, 3, 2]

    with tile.TileContext(nc) as tc:
        with ExitStack() as ctx:
            pers = ctx.enter_context(tc.tile_pool(name="pers", bufs=1))
            psum = ctx.enter_context(tc.tile_pool(name="psum", bufs=1, space="PSUM"))

            # Persistent across phases
            QT = pers.tile([P, NHP, T], BF, name="QT")       # rope'd Q^T
            KTs = pers.tile([P, NHP, T], BF, name="KTs")     # rope'd K^T
            VA = pers.tile([P, NT, HG, DH + 1], BF, name="VA")  # V' + ones col
            OT = pers.tile([P, NHP, T], BF, name="OT")       # normalized O^T

            nc.vector.memset(VA[:, :, :, DH:DH + 1], 1.0)

            # ---------------- Phase 1: projections + RoPE ----------------
            ph1 = ctx.enter_context(tc.tile_pool(name="work", bufs=1))
            if True:
                hid_sb = ph1.tile([P, KT, T], BF, name="hid_sb")
                for k in range(KT):
                    nc.sync.dma_start(hid_sb[:, k, :], hid_r[:, k, :])
                wv_sb = ph1.tile([P, KT, DG], BF, name="wv_sb")
                for k in range(KT):
                    nc.sync.dma_start(wv_sb[:, k, :], wv_r[:, k, :])
                cos_sb = ph1.tile([P, T], BF, name="cos_sb", tag="cos_sb")
                nc.sync.dma_start(cos_sb, cosT[:])
                sin_sb = ph1.tile([P, T], BF, name="sin_sb")
                nc.sync.dma_start(sin_sb, sinT[:])

                # V projection: V' = psum + lambda1*v1 (lambda2 folded into wv)
                for gs in range(0, NT, 3):
                    g = list(range(gs, min(gs + 3, NT)))
                    pt = psum.tile([P, 3, 512], F32, name="spsum", tag="spsum",
                                   bufs=2)
                    for idx, m in enumerate(g):
                        for k in range(KT):
                            nc.tensor.matmul(
                                pt[:, idx, :],
                                hid_sb[:, k, m * P:(m + 1) * P],
                                wv_sb[:, k, :],
                                start=(k == 0), stop=(k == KT - 1))
                    for idx, m in enumerate(g):
                        v1t = ph1.tile([P, 512], BF, name="v1t", tag="v1t", bufs=2)
                        nc.sync.dma_start(v1t, v1_r[:, m, :])
                        nc.vector.tensor_add(
                            VA[:, m, :, 0:DH],
                            pt[:, idx, :].rearrange("p (h d) -> p h d", h=HG),
                            v1t.rearrange("p (h d) -> p h d", h=HG))

                # Q^T / K^T projections + RoPE (transposed layout)
                # stream of psum chunks: (hp, which, n) n in 0..3 (t chunks)
                units = [(hp, w, n) for hp in range(NHP) for w in range(2)
                         for n in range(4)]
                pend = {}  # (hp, w) -> {n: psum slice}
                gi = 0
                while gi < len(units):
                    g = units[gi:gi + 3]
                    gi += 3
                    pt = psum.tile([P, 3, 512], F32, name="spsum", tag="spsum",
                                   bufs=2)
                    for idx, (hp, w, n) in enumerate(g):
                        w_sb = pend.get(("w", hp, w))
                        if w_sb is None:
                            w_sb = ph1.tile([P, KT, P], BF, name="wqk",
                                            tag="wqk", bufs=2)
                            src = wq_r if w == 0 else wk_r
                            nc.sync.dma_start(
                                w_sb, src[:, :, hp * P:(hp + 1) * P])
                            pend[("w", hp, w)] = w_sb
                        for k in range(KT):
                            nc.tensor.matmul(
                                pt[:, idx, :],
                                w_sb[:, k, :],
                                hid_sb[:, k, n * 512:(n + 1) * 512],
                                start=(k == 0), stop=(k == KT - 1))
                    for idx, (hp, w, n) in enumerate(g):
                        key = (hp, w, n // 2)
                        d = pend.setdefault(key, {})
                        d[n] = pt[:, idx, :]
                        if len(d) == 2:
                            # both 512-chunks of this 1024 half ready
                            h2 = n // 2
                            qtraw = ph1.tile([P, 1024], BF, name="qtraw",
                                             tag="qtraw", bufs=2)
                            for nn in sorted(d):
                                nc.scalar.copy(
                                    qtraw[:, (nn % 2) * 512:(nn % 2 + 1) * 512],
                                    d[nn])
                            qtswap = ph1.tile([P, 1024], BF, name="qtswap",
                                              tag="qtswap", bufs=2)
                            for blk in range(4):
                                sb = SWAP_SRC[blk]
                                nc.sync.dma_start(
                                    qtswap[blk * 32:(blk + 1) * 32, :],
                                    qtraw[sb * 32:(sb + 1) * 32, :])
                            ts = slice(h2 * 1024, (h2 + 1) * 1024)
                            dest = (QT if w == 0 else KTs)[:, hp, ts]
                            nc.vector.tensor_mul(dest, qtraw, cos_sb[:, ts])
                            nc.vector.tensor_mul(qtswap, qtswap, sin_sb[:, ts])
                            nc.vector.tensor_add(dest, dest, qtswap)
                            del pend[key]

            # ---------------- Phase 2+3: attention + output projection ----
            ph2 = ph1
            if True:
                mask_sb = ph2.tile([P, 4, 512], BF, name="mask_sb", tag="cos_sb")
                nc.sync.dma_start(mask_sb, maskp[:])
                wo_sb = ph2.tile([P, 4, T], BF, name="wo_sb", tag="wv_sb")
                for k4 in range(4):
                    nc.sync.dma_start(wo_sb[:, k4, :], wo_r[:, k4, :])

                for qc in range(NQC):
                    kmax = 4 * (qc + 1)
                    for hp in range(NHP):
                        osum = [psum.tile([65, 512], F32, name="osum",
                                          tag="osum", bufs=2)
                                for _ in range(2)]
                        qsl = slice(qc * 512, (qc + 1) * 512)
                        aunits = [(kt, hh) for kt in range(kmax)
                                  for hh in range(2)]
                        for gs in range(0, len(aunits), 3):
                            g = aunits[gs:gs + 3]
                            pt = psum.tile([P, 3, 512], F32, name="spsum",
                                           tag="spsum", bufs=2)
                            et = ph2.tile([P, 3, 512], BF, name="etile",
                                          tag="qtraw", bufs=2)
                            for idx, (kt, hh) in enumerate(g):
                                pb = hh * 64
                                nc.tensor.matmul(
                                    pt[:, idx, :],
                                    KTs[pb:pb + 64, hp,
                                        kt * P:(kt + 1) * P],
                                    QT[pb:pb + 64, hp, qsl],
                                    start=True, stop=True)
                            n = len(g)
                            nc.scalar.activation(
                                et[:, 0:n, :], pt[:, 0:n, :], AF.Exp,
                                scale=float(SCALE))
                            for idx, (kt, hh) in enumerate(g):
                                dd = kt - 4 * qc
                                if dd >= 0:
                                    nc.vector.tensor_mul(
                                        et[:, idx, :], et[:, idx, :],
                                        mask_sb[:, dd, :])
                                nc.tensor.matmul(
                                    osum[hh][:, :],
                                    VA[:, kt, hp * 2 + hh, 0:DH + 1],
                                    et[:, idx, :],
                                    start=(kt == 0), stop=(kt == kmax - 1))
                        for hh in range(2):
                            bcast = ph2.tile([64, 512], F32, name="bcast",
                                             tag="qtswap", bufs=2)
                            nc.vector.reciprocal(bcast[0:1, :],
                                                 osum[hh][64:65, :])
                            for db in range(6):
                                w = 1 << db
                                nc.sync.dma_start(bcast[w:2 * w, :],
                                                  bcast[0:w, :])
                            pb = hh * 64
                            nc.vector.tensor_mul(
                                OT[pb:pb + 64, hp, qsl],
                                osum[hh][0:64, :], bcast)

                    # output projection for this qc's four t tiles
                    for mt in range(qc * 4, qc * 4 + 4):
                        for gs in range(0, 4, 3):
                            g = list(range(gs, min(gs + 3, 4)))
                            pt = psum.tile([P, 3, 512], F32, name="spsum",
                                           tag="spsum", bufs=2)
                            for idx, n in enumerate(g):
                                for k4 in range(4):
                                    nc.tensor.matmul(
                                        pt[:, idx, :],
                                        OT[:, k4, mt * P:(mt + 1) * P],
                                        wo_sb[:, k4, n * 512:(n + 1) * 512],
                                        start=(k4 == 0), stop=(k4 == 3))
                            for idx, n in enumerate(g):
                                ost = ph2.tile([P, 512], F32, name="ostage",
                                               tag="v1t", bufs=2)
                                nc.vector.tensor_copy(ost, pt[:, idx, :])
                                nc.sync.dma_start(
                                    out_r[:, mt, n * 512:(n + 1) * 512], ost)

    nc.finalize()
    return nc


_NC = None


def _get_nc():
    global _NC
    if _NC is None:
        _NC = _build_bass()
    return _NC


def _make_in_maps(hidden_states, v1, lambda1, Wq, Wk, Wv, Wo, lambda2):
    cosT, sinT, mask = _CONSTS
    hidT_b = [np.ascontiguousarray(hidden_states[b].T).astype(BF16)
              for b in range(B)]
    wq_b = Wq.astype(BF16)
    wk_b = Wk.astype(BF16)
    wv_b = (np.float32(lambda2) * Wv).astype(BF16)
    wo_b = Wo.astype(BF16)
    v1l = (np.float32(lambda1) * v1).astype(BF16)

    in_maps = []
    for i in range(N_CORES):
        b, g = divmod(i, GROUPS)
        cs = slice(g * DG, (g + 1) * DG)
        in_maps.append({
            "hidT": hidT_b[b],
            "wq": np.ascontiguousarray(wq_b[:, cs]),
            "wk": np.ascontiguousarray(wk_b[:, cs]),
            "wv": np.ascontiguousarray(wv_b[:, cs]),
            "v1s": np.ascontiguousarray(
                v1l[b, :, g * HG:(g + 1) * HG, :]).reshape(T, DG),
            "cosT": cosT,
            "sinT": sinT,
            "mask": mask,
            "wo": np.ascontiguousarray(wo_b[cs, :]),
        })
    return in_maps


def kernel(hidden_states, v1, lambda1, Wq, Wk, Wv, Wo, lambda2):
    from concourse.bass_utils import run_bass_kernel_spmd

    args = (np.asarray(hidden_states, np.float32), np.asarray(v1, np.float32),
            np.float32(lambda1), np.asarray(Wq, np.float32),
            np.asarray(Wk, np.float32), np.asarray(Wv, np.float32),
            np.asarray(Wo, np.float32), np.float32(lambda2))
    nc = _get_nc()
    in_maps = _make_in_maps(*args)
    res = run_bass_kernel_spmd(nc, in_maps, core_ids=list(range(N_CORES)))
    parts = [r["out"] for r in res.results]
    out = np.empty((B, T, DM), np.float32)
    for b in range(B):
        out[b] = parts[b * GROUPS]
        for g in range(1, GROUPS):
            out[b] += parts[b * GROUPS + g]
    return out


# revision 11
# speedup vs baseline: 1.3424x; 1.3424x over previous
"""Bass/Tile kernel for nn_Attention_41532333753073 on 8 trn2 NeuronCores.

Sharding: core i -> (batch b = i//4, head-group g = i%4) with 8 heads each.
Per core: QKV projections (bf16 matmuls, fp32 psum), RoPE, causal attention in
S^T layout (k_t on partitions -> softmax row-sum via ones-column in the PV
matmul, no transposes), output projection producing a partial [T, DM] that the
host sums over the 4 head-groups per batch.
"""

import numpy as np
import ml_dtypes

BF16 = ml_dtypes.bfloat16

# Problem shapes (hardcoded)
B, T, DM = 2, 2048, 2048
H, DH = 32, 64
GROUPS = 4
HG = H // GROUPS          # 8 heads per group
DG = HG * DH              # 512 columns per group
P = 128
KT = DM // P              # 16 contraction tiles
NT = T // P               # 16 t tiles
NHP = HG // 2             # 4 head pairs
NQC = T // 512            # 4 q chunks of 512
ROPE_THETA = 10000.0
N_CORES = 8
SCALE = 1.0 / np.sqrt(DH)


def _host_constants():
    inv = 1.0 / (ROPE_THETA ** (np.arange(0, DH, 2, dtype=np.float64) / DH))  # [32]
    t = np.arange(T, dtype=np.float64)
    f = np.outer(inv, t)                       # [32, T]
    cos = np.cos(f).astype(np.float32)
    sin = np.sin(f).astype(np.float32)
    cosT = np.tile(cos, (4, 1))                # [128, T]
    sgn = np.repeat(np.array([-1.0, 1.0, -1.0, 1.0], np.float32), 32)[:, None]
    sinT = np.tile(sin, (4, 1)) * sgn          # [128, T] signed
    i = np.arange(P)[:, None]
    j = np.arange(512)[None, :]
    mask = np.stack([(j - i >= 128 * d) for d in range(4)], axis=1)  # [128,4,512]
    return (cosT.astype(BF16), sinT.astype(BF16), mask.astype(BF16))


_CONSTS = _host_constants()

_SEG_SIZES = [
    ("hidT", DM * T),
    ("wq", DM * DG),
    ("wk", DM * DG),
    ("wv", DM * DG),
    ("v1s", T * DG),
    ("cosT", P * T),
    ("sinT", P * T),
    ("mask", P * 4 * 512),
    ("wo", DG * DM),
]
_SEG = {}
_off = 0
for _n, _s in _SEG_SIZES:
    _SEG[_n] = (_off, _off + _s)
    _off += _s
BLOB_ELEMS = _off




def _build_bass():
    from contextlib import ExitStack

    import concourse.bass as bass
    import concourse.bacc as bacc
    import concourse.tile as tile
    from concourse import mybir

    BF = mybir.dt.bfloat16
    F32 = mybir.dt.float32
    AF = mybir.ActivationFunctionType

    nc = bacc.Bacc(None, target_bir_lowering=False)
    blob = nc.declare_dram_parameter("blob", [BLOB_ELEMS], BF, isOutput=False)
    out = nc.declare_dram_parameter("out", [T, DM], BF, isOutput=True)

    bp = blob[:]

    def seg(name, pat, **kw):
        o0, o1 = _SEG[name]
        return bp[o0:o1].rearrange(pat, **kw)

    hid_r = seg("hidT", "(k p t) -> p k t", p=P, t=T)      # [128,16,2048]
    wq_r = seg("wq", "(k p c) -> p k c", p=P, c=DG)        # [128,16,512]
    wk_r = seg("wk", "(k p c) -> p k c", p=P, c=DG)
    wv_r = seg("wv", "(k p c) -> p k c", p=P, c=DG)
    v1_r = seg("v1s", "(n p c) -> p n c", p=P, c=DG)       # [128,16,512]
    cosT = seg("cosT", "(p t) -> p t", p=P)
    sinT = seg("sinT", "(p t) -> p t", p=P)
    maskp = seg("mask", "(p d c) -> p d c", p=P, d=4)
    wo_r = seg("wo", "(k p n) -> p k n", p=P, n=DM)        # [128,4,2048]
    out_r = out[:].rearrange("(m p) n -> p m n", p=P)      # [128,16,2048]

    SWAP_SRC = [1, 0, 3, 2]

    with tile.TileContext(nc) as tc:
        with ExitStack() as ctx:
            pers = ctx.enter_context(tc.tile_pool(name="pers", bufs=1))
            psum = ctx.enter_context(tc.tile_pool(name="psum", bufs=1, space="PSUM"))

            # Persistent across phases
            QT = pers.tile([P, NHP, T], BF, name="QT")       # rope'd Q^T
            KTs = pers.tile([P, NHP, T], BF, name="KTs")     # rope'd K^T
            VA = pers.tile([P, NT, HG, DH + 1], BF, name="VA")  # V' + ones col
            OT = pers.tile([P, NHP, T], BF, name="OT")       # normalized O^T

            nc.vector.memset(VA[:, :, :, DH:DH + 1], 1.0)

            # ---------------- Phase 1: projections + RoPE ----------------
            ph1 = ctx.enter_context(tc.tile_pool(name="work", bufs=1))
            if True:
                hid_sb = ph1.tile([P, KT, T], BF, name="hid_sb")
                for k in range(KT):
                    nc.sync.dma_start(hid_sb[:, k, :], hid_r[:, k, :])
                wv_sb = ph1.tile([P, KT, DG], BF, name="wv_sb")
                for k in range(KT):
                    nc.sync.dma_start(wv_sb[:, k, :], wv_r[:, k, :])
                cos_sb = ph1.tile([P, T], BF, name="cos_sb", tag="cos_sb")
                nc.sync.dma_start(cos_sb, cosT)
                sin_sb = ph1.tile([P, T], BF, name="sin_sb")
                nc.sync.dma_start(sin_sb, sinT)

                # V projection: V' = psum + lambda1*v1 (lambda2 folded into wv)
                for gs in range(0, NT, 3):
                    g = list(range(gs, min(gs + 3, NT)))
                    pt = psum.tile([P, 3, 512], F32, name="spsum", tag="spsum",
                                   bufs=2)
                    for idx, m in enumerate(g):
                        for k in range(KT):
                            nc.tensor.matmul(
                                pt[:, idx, :],
                                hid_sb[:, k, m * P:(m + 1) * P],
                                wv_sb[:, k, :],
                                start=(k == 0), stop=(k == KT - 1))
                    for idx, m in enumerate(g):
                        v1t = ph1.tile([P, 512], BF, name="v1t", tag="v1t", bufs=2)
                        nc.sync.dma_start(v1t, v1_r[:, m, :])
                        nc.vector.tensor_add(
                            VA[:, m, :, 0:DH],
                            pt[:, idx, :].rearrange("p (h d) -> p h d", h=HG),
                            v1t.rearrange("p (h d) -> p h d", h=HG))

                # Q^T / K^T projections + RoPE (transposed layout)
                # stream of psum chunks: (hp, which, n) n in 0..3 (t chunks)
                units = [(hp, w, n) for hp in range(NHP) for w in range(2)
                         for n in range(4)]
                pend = {}  # (hp, w) -> {n: psum slice}
                gi = 0
                while gi < len(units):
                    g = units[gi:gi + 3]
                    gi += 3
                    pt = psum.tile([P, 3, 512], F32, name="spsum", tag="spsum",
                                   bufs=2)
                    for idx, (hp, w, n) in enumerate(g):
                        w_sb = pend.get(("w", hp, w))
                        if w_sb is None:
                            w_sb = ph1.tile([P, KT, P], BF, name="wqk",
                                            tag="wqk", bufs=2)
                            src = wq_r if w == 0 else wk_r
                            nc.sync.dma_start(
                                w_sb, src[:, :, hp * P:(hp + 1) * P])
                            pend[("w", hp, w)] = w_sb
                        for k in range(KT):
                            nc.tensor.matmul(
                                pt[:, idx, :],
                                w_sb[:, k, :],
                                hid_sb[:, k, n * 512:(n + 1) * 512],
                                start=(k == 0), stop=(k == KT - 1))
                    for idx, (hp, w, n) in enumerate(g):
                        key = (hp, w, n // 2)
                        d = pend.setdefault(key, {})
                        d[n] = pt[:, idx, :]
                        if len(d) == 2:
                            # both 512-chunks of this 1024 half ready
                            h2 = n // 2
                            qtraw = ph1.tile([P, 1024], BF, name="qtraw",
                                             tag="qtraw", bufs=2)
                            for nn in sorted(d):
                                nc.scalar.copy(
                                    qtraw[:, (nn % 2) * 512:(nn % 2 + 1) * 512],
                                    d[nn])
                            qtswap = ph1.tile([P, 1024], BF, name="qtswap",
                                              tag="qtswap", bufs=2)
                            for blk in range(4):
                                sb = SWAP_SRC[blk]
                                nc.sync.dma_start(
                                    qtswap[blk * 32:(blk + 1) * 32, :],
                                    qtraw[sb * 32:(sb + 1) * 32, :])
                            ts = slice(h2 * 1024, (h2 + 1) * 1024)
                            dest = (QT if w == 0 else KTs)[:, hp, ts]
                            nc.vector.tensor_mul(dest, qtraw, cos_sb[:, ts])
                            nc.vector.tensor_mul(qtswap, qtswap, sin_sb[:, ts])
                            nc.vector.tensor_add(dest, dest, qtswap)
                            del pend[key]

            # ---------------- Phase 2+3: attention + output projection ----
            ph2 = ph1
            if True:
                mask_sb = ph2.tile([P, 4, 512], BF, name="mask_sb", tag="cos_sb")
                nc.sync.dma_start(mask_sb, maskp)
                wo_sb = ph2.tile([P, 4, T], BF, name="wo_sb", tag="wv_sb")
                for k4 in range(4):
                    nc.sync.dma_start(wo_sb[:, k4, :], wo_r[:, k4, :])

                for qc in range(NQC):
                    kmax = 4 * (qc + 1)
                    for hp in range(NHP):
                        osum = [psum.tile([65, 512], F32, name="osum",
                                          tag="osum", bufs=2)
                                for _ in range(2)]
                        qsl = slice(qc * 512, (qc + 1) * 512)
                        aunits = [(kt, hh) for kt in range(kmax)
                                  for hh in range(2)]
                        for gs in range(0, len(aunits), 3):
                            g = aunits[gs:gs + 3]
                            pt = psum.tile([P, 3, 512], F32, name="spsum",
                                           tag="spsum", bufs=2)
                            et = ph2.tile([P, 3, 512], BF, name="etile",
                                          tag="qtraw", bufs=2)
                            for idx, (kt, hh) in enumerate(g):
                                pb = hh * 64
                                nc.tensor.matmul(
                                    pt[:, idx, :],
                                    KTs[pb:pb + 64, hp,
                                        kt * P:(kt + 1) * P],
                                    QT[pb:pb + 64, hp, qsl],
                                    start=True, stop=True)
                            n = len(g)
                            nc.scalar.activation(
                                et[:, 0:n, :], pt[:, 0:n, :], AF.Exp,
                                scale=float(SCALE))
                            for idx, (kt, hh) in enumerate(g):
                                dd = kt - 4 * qc
                                if dd >= 0:
                                    nc.vector.tensor_mul(
                                        et[:, idx, :], et[:, idx, :],
                                        mask_sb[:, dd, :])
                                nc.tensor.matmul(
                                    osum[hh][:, :],
                                    VA[:, kt, hp * 2 + hh, 0:DH + 1],
                                    et[:, idx, :],
                                    start=(kt == 0), stop=(kt == kmax - 1))
                        for hh in range(2):
                            bcast = ph2.tile([64, 512], F32, name="bcast",
                                             tag="qtswap", bufs=2)
                            nc.vector.reciprocal(bcast[0:1, :],
                                                 osum[hh][64:65, :])
                            for db in range(6):
                                w = 1 << db
                                nc.sync.dma_start(bcast[w:2 * w, :],
                                                  bcast[0:w, :])
                            pb = hh * 64
                            nc.vector.tensor_mul(
                                OT[pb:pb + 64, hp, qsl],
                                osum[hh][0:64, :], bcast)

                    # output projection for this qc's four t tiles
                    for mt in range(qc * 4, qc * 4 + 4):
                        for gs in range(0, 4, 3):
                            g = list(range(gs, min(gs + 3, 4)))
                            pt = psum.tile([P, 3, 512], F32, name="spsum",
                                           tag="spsum", bufs=2)
                            for idx, n in enumerate(g):
                                for k4 in range(4):
                                    nc.tensor.matmul(
                                        pt[:, idx, :],
                                        OT[:, k4, mt * P:(mt + 1) * P],
                                        wo_sb[:, k4, n * 512:(n + 1) * 512],
                                        start=(k4 == 0), stop=(k4 == 3))
                            for idx, n in enumerate(g):
                                ost = ph2.tile([P, 512], BF, name="ostage",
                                               tag="v1t", bufs=2)
                                nc.vector.tensor_copy(ost, pt[:, idx, :])
                                nc.sync.dma_start(
                                    out_r[:, mt, n * 512:(n + 1) * 512], ost)

    nc.finalize()
    return nc


_NC = None


def _get_nc():
    global _NC
    if _NC is None:
        _NC = _build_bass()
    return _NC


def _make_in_maps(hidden_states, v1, lambda1, Wq, Wk, Wv, Wo, lambda2):
    cosT, sinT, mask = _CONSTS
    hidT_b = [np.ascontiguousarray(hidden_states[b].T).astype(BF16)
              for b in range(B)]
    wq_b = Wq.astype(BF16)
    wk_b = Wk.astype(BF16)
    wv_b = (np.float32(lambda2) * Wv).astype(BF16)
    wo_b = Wo.astype(BF16)
    v1l = (np.float32(lambda1) * v1).astype(BF16)

    in_maps = []
    for i in range(N_CORES):
        b, g = divmod(i, GROUPS)
        cs = slice(g * DG, (g + 1) * DG)
        parts = [
            hidT_b[b].ravel(),
            wq_b[:, cs].ravel(),
            wk_b[:, cs].ravel(),
            wv_b[:, cs].ravel(),
            v1l[b, :, g * HG:(g + 1) * HG, :].ravel(),
            cosT.ravel(),
            sinT.ravel(),
            mask.ravel(),
            wo_b[cs, :].ravel(),
        ]
        in_maps.append({"blob": np.concatenate(parts)})
    return in_maps


def kernel(hidden_states, v1, lambda1, Wq, Wk, Wv, Wo, lambda2):
    from concourse.bass_utils import run_bass_kernel_spmd

    args = (np.asarray(hidden_states, np.float32), np.asarray(v1, np.float32),
            np.float32(lambda1), np.asarray(Wq, np.float32),
            np.asarray(Wk, np.float32), np.asarray(Wv, np.float32),
            np.asarray(Wo, np.float32), np.float32(lambda2))
    nc = _get_nc()
    in_maps = _make_in_maps(*args)
    res = run_bass_kernel_spmd(nc, in_maps, core_ids=list(range(N_CORES)))
    parts = [r["out"] for r in res.results]
    out = np.empty((B, T, DM), np.float32)
    for b in range(B):
        out[b] = parts[b * GROUPS].astype(np.float32)
        for g in range(1, GROUPS):
            out[b] += parts[b * GROUPS + g].astype(np.float32)
    return out


# revision 13
# speedup vs baseline: 3.1928x; 2.3784x over previous
"""Bass/Tile kernel for nn_Attention_41532333753073 on 8 trn2 NeuronCores.

Sharding: core i -> (batch b = i//4, head-group g = i%4) with 8 heads each.
Per core: QKV projections (bf16 matmuls, fp32 psum), RoPE, causal attention in
S^T layout (k_t on partitions -> softmax row-sum via ones-column in the PV
matmul, no transposes), output projection producing a partial [T, DM] that the
host sums over the 4 head-groups per batch.
"""

import numpy as np
import ml_dtypes

BF16 = ml_dtypes.bfloat16

# Problem shapes (hardcoded)
B, T, DM = 2, 2048, 2048
H, DH = 32, 64
GROUPS = 4
HG = H // GROUPS          # 8 heads per group
DG = HG * DH              # 512 columns per group
P = 128
KT = DM // P              # 16 contraction tiles
NT = T // P               # 16 t tiles
NHP = HG // 2             # 4 head pairs
NQC = T // 512            # 4 q chunks of 512
ROPE_THETA = 10000.0
N_CORES = 8
SCALE = 1.0 / np.sqrt(DH)


def _host_constants():
    inv = 1.0 / (ROPE_THETA ** (np.arange(0, DH, 2, dtype=np.float64) / DH))  # [32]
    t = np.arange(T, dtype=np.float64)
    f = np.outer(inv, t)                       # [32, T]
    cos = np.cos(f).astype(np.float32)
    sin = np.sin(f).astype(np.float32)
    cosT = np.tile(cos, (4, 1))                # [128, T]
    sgn = np.repeat(np.array([-1.0, 1.0, -1.0, 1.0], np.float32), 32)[:, None]
    sinT = np.tile(sin, (4, 1)) * sgn          # [128, T] signed
    i = np.arange(P)[:, None]
    j = np.arange(512)[None, :]
    mask = np.stack([(j - i >= 128 * d) for d in range(4)], axis=1)  # [128,4,512]
    return (cosT.astype(BF16), sinT.astype(BF16), mask.astype(BF16))


_CONSTS = _host_constants()

_W1 = DM * DG          # one [2048,512] weight, elements
HID_SLICE = 512 * T    # per-core hidT row-slice
W_HALF = 2 * _W1       # (wq|wk) for b=0, (wv|wo) for b=1
CONSTS = 2 * P * T + P * 4 * 512   # cos | sin | mask
CSLICE = CONSTS // 8
V1SEG = T * DG

# per-core blob layout
_SEG = {}
_off = 0
for _n, _s in [("hid_slice", HID_SLICE), ("w_half", W_HALF),
               ("c_slice", CSLICE), ("v1s", V1SEG)]:
    _SEG[_n] = (_off, _off + _s)
    _off += _s
BLOB_ELEMS = _off

# gathered-weights layout: wq | wk | wv | wo
_WSEG = {"wq": (0, _W1), "wk": (_W1, 2 * _W1), "wv": (2 * _W1, 3 * _W1),
         "wo": (3 * _W1, 4 * _W1)}
# gathered-consts layout
_CSEG = {"cosT": (0, P * T), "sinT": (P * T, 2 * P * T),
         "mask": (2 * P * T, CONSTS)}


def _build_bass():
    from contextlib import ExitStack

    import concourse.bass as bass
    import concourse.bacc as bacc
    import concourse.tile as tile
    from concourse import mybir

    BF = mybir.dt.bfloat16
    F32 = mybir.dt.float32
    AF = mybir.ActivationFunctionType

    nc = bacc.Bacc(None, target_bir_lowering=False)
    blob = nc.declare_dram_parameter("blob", [BLOB_ELEMS], BF, isOutput=False)
    out = nc.declare_dram_parameter("out", [512, DM], BF, isOutput=True)

    bp = blob[:]
    out_r = out[:].rearrange("(m p) n -> p m n", p=P)      # [128,4,2048]

    SWAP_SRC = [1, 0, 3, 2]

    G_BATCH = [[0, 1, 2, 3], [4, 5, 6, 7]]
    G_PAIR = [[0, 4], [1, 5], [2, 6], [3, 7]]
    G_ALL = [[0, 1, 2, 3, 4, 5, 6, 7]]

    with tile.TileContext(nc) as tc:
        with ExitStack() as ctx:
            pers = ctx.enter_context(tc.tile_pool(name="pers", bufs=1))
            psum = ctx.enter_context(tc.tile_pool(name="psum", bufs=1, space="PSUM"))
            dram = ctx.enter_context(tc.tile_pool(name="dram", bufs=1,
                                                  space="DRAM"))

            # gather full inputs from per-core slices
            cin_hid = dram.tile([HID_SLICE], BF, name="cin_hid")
            ag_hid = dram.tile([4 * HID_SLICE], BF, name="ag_hid")
            s0, s1 = _SEG["hid_slice"]
            nc.sync.dma_start(cin_hid[:], bp[s0:s1])
            nc.gpsimd.collective_compute(
                "AllGather", mybir.AluOpType.bypass, replica_groups=G_BATCH,
                ins=[cin_hid[:]], outs=[ag_hid[:]])

            cin_w = dram.tile([W_HALF], BF, name="cin_w")
            ag_w = dram.tile([2 * W_HALF], BF, name="ag_w")
            s0, s1 = _SEG["w_half"]
            nc.sync.dma_start(cin_w[:], bp[s0:s1])
            nc.gpsimd.collective_compute(
                "AllGather", mybir.AluOpType.bypass, replica_groups=G_PAIR,
                ins=[cin_w[:]], outs=[ag_w[:]])

            cin_c = dram.tile([CSLICE], BF, name="cin_c")
            ag_c = dram.tile([CONSTS], BF, name="ag_c")
            s0, s1 = _SEG["c_slice"]
            nc.sync.dma_start(cin_c[:], bp[s0:s1])
            nc.gpsimd.collective_compute(
                "AllGather", mybir.AluOpType.bypass, replica_groups=G_ALL,
                ins=[cin_c[:]], outs=[ag_c[:]])

            pout = dram.tile([T, DM], F32, name="pout")
            rsout = dram.tile([512, DM], F32, name="rsout")

            def wseg(name, pat, **kw):
                o0, o1 = _WSEG[name]
                return ag_w[o0:o1].rearrange(pat, **kw)

            def cseg(name, pat, **kw):
                o0, o1 = _CSEG[name]
                return ag_c[o0:o1].rearrange(pat, **kw)

            hid_r = ag_hid[:].rearrange("(k p t) -> p k t", p=P, t=T)
            wq_r = wseg("wq", "(k p c) -> p k c", p=P, c=DG)
            wk_r = wseg("wk", "(k p c) -> p k c", p=P, c=DG)
            wv_r = wseg("wv", "(k p c) -> p k c", p=P, c=DG)
            wo_r = wseg("wo", "(k p n) -> p k n", p=P, n=DM)
            cosT = cseg("cosT", "(p t) -> p t", p=P)
            sinT = cseg("sinT", "(p t) -> p t", p=P)
            maskp = cseg("mask", "(p d c) -> p d c", p=P, d=4)
            s0, s1 = _SEG["v1s"]
            v1_r = bp[s0:s1].rearrange("(n p c) -> p n c", p=P, c=DG)
            pout_r = pout[:].rearrange("(m p) n -> p m n", p=P)

            # Persistent across phases
            QT = pers.tile([P, NHP, T], BF, name="QT")       # rope'd Q^T
            KTs = pers.tile([P, NHP, T], BF, name="KTs")     # rope'd K^T
            VA = pers.tile([P, NT, HG, DH + 1], BF, name="VA")  # V' + ones col
            OT = pers.tile([P, NHP, T], BF, name="OT")       # normalized O^T

            nc.vector.memset(VA[:, :, :, DH:DH + 1], 1.0)

            # ---------------- Phase 1: projections + RoPE ----------------
            ph1 = ctx.enter_context(tc.tile_pool(name="work", bufs=1))
            if True:
                hid_sb = ph1.tile([P, KT, T], BF, name="hid_sb")
                for k in range(KT):
                    nc.sync.dma_start(hid_sb[:, k, :], hid_r[:, k, :])
                wv_sb = ph1.tile([P, KT, DG], BF, name="wv_sb")
                for k in range(KT):
                    nc.sync.dma_start(wv_sb[:, k, :], wv_r[:, k, :])
                cos_sb = ph1.tile([P, T], BF, name="cos_sb", tag="cos_sb")
                nc.sync.dma_start(cos_sb, cosT)
                sin_sb = ph1.tile([P, T], BF, name="sin_sb")
                nc.sync.dma_start(sin_sb, sinT)

                # V projection: V' = psum + lambda1*v1 (lambda2 folded into wv)
                for gs in range(0, NT, 3):
                    g = list(range(gs, min(gs + 3, NT)))
                    pt = psum.tile([P, 3, 512], F32, name="spsum", tag="spsum",
                                   bufs=2)
                    for idx, m in enumerate(g):
                        for k in range(KT):
                            nc.tensor.matmul(
                                pt[:, idx, :],
                                hid_sb[:, k, m * P:(m + 1) * P],
                                wv_sb[:, k, :],
                                start=(k == 0), stop=(k == KT - 1))
                    for idx, m in enumerate(g):
                        v1t = ph1.tile([P, 512], BF, name="v1t", tag="v1t", bufs=2)
                        nc.sync.dma_start(v1t, v1_r[:, m, :])
                        nc.vector.tensor_add(
                            VA[:, m, :, 0:DH],
                            pt[:, idx, :].rearrange("p (h d) -> p h d", h=HG),
                            v1t.rearrange("p (h d) -> p h d", h=HG))

                # Q^T / K^T projections + RoPE (transposed layout)
                # stream of psum chunks: (hp, which, n) n in 0..3 (t chunks)
                units = [(hp, w, n) for hp in range(NHP) for w in range(2)
                         for n in range(4)]
                pend = {}  # (hp, w) -> {n: psum slice}
                gi = 0
                while gi < len(units):
                    g = units[gi:gi + 3]
                    gi += 3
                    pt = psum.tile([P, 3, 512], F32, name="spsum", tag="spsum",
                                   bufs=2)
                    for idx, (hp, w, n) in enumerate(g):
                        w_sb = pend.get(("w", hp, w))
                        if w_sb is None:
                            w_sb = ph1.tile([P, KT, P], BF, name="wqk",
                                            tag="wqk", bufs=2)
                            src = wq_r if w == 0 else wk_r
                            nc.sync.dma_start(
                                w_sb, src[:, :, hp * P:(hp + 1) * P])
                            pend[("w", hp, w)] = w_sb
                        for k in range(KT):
                            nc.tensor.matmul(
                                pt[:, idx, :],
                                w_sb[:, k, :],
                                hid_sb[:, k, n * 512:(n + 1) * 512],
                                start=(k == 0), stop=(k == KT - 1))
                    for idx, (hp, w, n) in enumerate(g):
                        key = (hp, w, n // 2)
                        d = pend.setdefault(key, {})
                        d[n] = pt[:, idx, :]
                        if len(d) == 2:
                            # both 512-chunks of this 1024 half ready
                            h2 = n // 2
                            qtraw = ph1.tile([P, 1024], BF, name="qtraw",
                                             tag="qtraw", bufs=2)
                            for nn in sorted(d):
                                nc.scalar.copy(
                                    qtraw[:, (nn % 2) * 512:(nn % 2 + 1) * 512],
                                    d[nn])
                            qtswap = ph1.tile([P, 1024], BF, name="qtswap",
                                              tag="qtswap", bufs=2)
                            for blk in range(4):
                                sb = SWAP_SRC[blk]
                                nc.sync.dma_start(
                                    qtswap[blk * 32:(blk + 1) * 32, :],
                                    qtraw[sb * 32:(sb + 1) * 32, :])
                            ts = slice(h2 * 1024, (h2 + 1) * 1024)
                            dest = (QT if w == 0 else KTs)[:, hp, ts]
                            nc.vector.tensor_mul(dest, qtraw, cos_sb[:, ts])
                            nc.vector.tensor_mul(qtswap, qtswap, sin_sb[:, ts])
                            nc.vector.tensor_add(dest, dest, qtswap)
                            del pend[key]

            # ---------------- Phase 2+3: attention + output projection ----
            ph2 = ph1
            if True:
                mask_sb = ph2.tile([P, 4, 512], BF, name="mask_sb", tag="cos_sb")
                nc.sync.dma_start(mask_sb, maskp)
                wo_sb = ph2.tile([P, 4, T], BF, name="wo_sb", tag="wv_sb")
                for k4 in range(4):
                    nc.sync.dma_start(wo_sb[:, k4, :], wo_r[:, k4, :])

                for qc in range(NQC):
                    kmax = 4 * (qc + 1)
                    for hp in range(NHP):
                        osum = [psum.tile([65, 512], F32, name="osum",
                                          tag="osum", bufs=2)
                                for _ in range(2)]
                        qsl = slice(qc * 512, (qc + 1) * 512)
                        aunits = [(kt, hh) for kt in range(kmax)
                                  for hh in range(2)]
                        for gs in range(0, len(aunits), 3):
                            g = aunits[gs:gs + 3]
                            pt = psum.tile([P, 3, 512], F32, name="spsum",
                                           tag="spsum", bufs=2)
                            et = ph2.tile([P, 3, 512], BF, name="etile",
                                          tag="qtraw", bufs=2)
                            for idx, (kt, hh) in enumerate(g):
                                pb = hh * 64
                                nc.tensor.matmul(
                                    pt[:, idx, :],
                                    KTs[pb:pb + 64, hp,
                                        kt * P:(kt + 1) * P],
                                    QT[pb:pb + 64, hp, qsl],
                                    start=True, stop=True)
                            n = len(g)
                            nc.scalar.activation(
                                et[:, 0:n, :], pt[:, 0:n, :], AF.Exp,
                                scale=float(SCALE))
                            for idx, (kt, hh) in enumerate(g):
                                dd = kt - 4 * qc
                                if dd >= 0:
                                    nc.vector.tensor_mul(
                                        et[:, idx, :], et[:, idx, :],
                                        mask_sb[:, dd, :])
                                nc.tensor.matmul(
                                    osum[hh][:, :],
                                    VA[:, kt, hp * 2 + hh, 0:DH + 1],
                                    et[:, idx, :],
                                    start=(kt == 0), stop=(kt == kmax - 1))
                        for hh in range(2):
                            bcast = ph2.tile([64, 512], F32, name="bcast",
                                             tag="qtswap", bufs=2)
                            nc.vector.reciprocal(bcast[0:1, :],
                                                 osum[hh][64:65, :])
                            for db in range(6):
                                w = 1 << db
                                nc.sync.dma_start(bcast[w:2 * w, :],
                                                  bcast[0:w, :])
                            pb = hh * 64
                            nc.vector.tensor_mul(
                                OT[pb:pb + 64, hp, qsl],
                                osum[hh][0:64, :], bcast)

                    # output projection for this qc's four t tiles
                    for mt in range(qc * 4, qc * 4 + 4):
                        for gs in range(0, 4, 3):
                            g = list(range(gs, min(gs + 3, 4)))
                            pt = psum.tile([P, 3, 512], F32, name="spsum",
                                           tag="spsum", bufs=2)
                            for idx, n in enumerate(g):
                                for k4 in range(4):
                                    nc.tensor.matmul(
                                        pt[:, idx, :],
                                        OT[:, k4, mt * P:(mt + 1) * P],
                                        wo_sb[:, k4, n * 512:(n + 1) * 512],
                                        start=(k4 == 0), stop=(k4 == 3))
                            for idx, n in enumerate(g):
                                ost = ph2.tile([P, 512], F32, name="ostage",
                                               tag="v1t", bufs=2)
                                nc.vector.tensor_copy(ost, pt[:, idx, :])
                                nc.sync.dma_start(
                                    pout_r[:, mt, n * 512:(n + 1) * 512], ost)

                # reduce partials across the 4-core batch group; each core
                # keeps its quarter of the rows, then emits it in bf16
                if qc == NQC - 1:
                    nc.gpsimd.collective_compute(
                        "ReduceScatter", mybir.AluOpType.add,
                        replica_groups=G_BATCH,
                        ins=[pout[:]], outs=[rsout[:]])
                    rs_r = rsout[:].rearrange("(m p) n -> p m n", p=P)
                    for m4 in range(4):
                        fin = ph2.tile([P, T], F32, name="fin", tag="qtraw",
                                       bufs=2)
                        nc.sync.dma_start(fin, rs_r[:, m4, :])
                        finb = ph2.tile([P, T], BF, name="finb", tag="qtswap",
                                        bufs=2)
                        nc.vector.tensor_copy(finb, fin)
                        nc.sync.dma_start(out_r[:, m4, :], finb)

    nc.finalize()
    return nc


_NC = None


def _get_nc():
    global _NC
    if _NC is None:
        _NC = _build_bass()
    return _NC


def _make_in_maps(hidden_states, v1, lambda1, Wq, Wk, Wv, Wo, lambda2):
    cosT, sinT, mask = _CONSTS
    consts = np.concatenate([cosT.ravel(), sinT.ravel(), mask.ravel()])
    hidT_b = [np.ascontiguousarray(hidden_states[b].T).astype(BF16)
              for b in range(B)]
    wq_b = Wq.astype(BF16)
    wk_b = Wk.astype(BF16)
    wv_b = (np.float32(lambda2) * Wv).astype(BF16)
    wo_b = Wo.astype(BF16)
    v1l = (np.float32(lambda1) * v1).astype(BF16)

    in_maps = []
    for i in range(N_CORES):
        b, g = divmod(i, GROUPS)
        cs = slice(g * DG, (g + 1) * DG)
        if b == 0:
            w_half = [wq_b[:, cs].ravel(), wk_b[:, cs].ravel()]
        else:
            w_half = [wv_b[:, cs].ravel(), wo_b[cs, :].ravel()]
        parts = [
            hidT_b[b][g * 512:(g + 1) * 512, :].ravel(),
            *w_half,
            consts[i * CSLICE:(i + 1) * CSLICE],
            v1l[b, :, g * HG:(g + 1) * HG, :].ravel(),
        ]
        in_maps.append({"blob": np.concatenate(parts)})
    return in_maps


def kernel(hidden_states, v1, lambda1, Wq, Wk, Wv, Wo, lambda2):
    from concourse.bass_utils import run_bass_kernel_spmd

    args = (np.asarray(hidden_states, np.float32), np.asarray(v1, np.float32),
            np.float32(lambda1), np.asarray(Wq, np.float32),
            np.asarray(Wk, np.float32), np.asarray(Wv, np.float32),
            np.asarray(Wo, np.float32), np.float32(lambda2))
    nc = _get_nc()
    in_maps = _make_in_maps(*args)
    res = run_bass_kernel_spmd(nc, in_maps, core_ids=list(range(N_CORES)))
    parts = [r["out"] for r in res.results]
    out = np.empty((B, T, DM), np.float32)
    for b in range(B):
        for j in range(GROUPS):
            out[b, j * 512:(j + 1) * 512] = parts[b * GROUPS + j]
    return out
